# revision 35
# baseline (speedup 1.0000x reference)
"""Trainium2 Bass kernel for nn_FRMIL (dense_transformer, 8 NeuronCores).

Pipeline (2 SPMD launches over 8 cores):
  Launch A: per-core enc scoring a1 = sigmoid(x @ w_enc + b_enc), top-1 value+index.
            Host picks the global argmax row -> Q.
  Launch B: per-core (grid-row sharded, sequence-parallel):
            xs = relu(x - Q)  [fp32, streamed out as i_shift]
            depthwise 3x3 conv over the 245x245 grid (+bias+residual) -> xx (bf16)
            scores_h = (w_k @ q_heads).T @ xx / sqrt(D) (+ q.b_k)
            online softmax over positions: m, sumexp, T = sum_s p_s * xx[:, s]
  Host: combines per-core (m, sumexp, T) partials, applies the tiny w_v/w_o/w_fc
        tail (~1 MFLOP), assembles full outputs.

Layout: "B" = feature-on-partition [512 -> 4 chunks of 128, positions on free axis].
Grid rows padded to width 247 (zero col left/right) so conv col-taps are free-axis
shifts; grid-row halo rows come via overlapping DMA windows.
"""
import numpy as np
import ml_dtypes

import concourse.bacc as bacc
import concourse.bass as bass
import concourse.tile as tile
import concourse.mybir as mybir
from concourse.bass_utils import run_bass_kernel_spmd

f32 = mybir.dt.float32
f32r = mybir.dt.float32r
bf16 = mybir.dt.bfloat16
u32 = mybir.dt.uint32
AF = mybir.ActivationFunctionType
ALU = mybir.AluOpType
BF = ml_dtypes.bfloat16

N, D, NH, DH, NCLS = 60000, 512, 8, 64, 2
GR = 245                 # grid rows/cols
WP = 247                 # padded row width
NC = 8                   # cores
NA = N // NC             # 7500 rows/core for launch A
NAP = 7680               # padded to 15*512
ROWS = 31                # grid rows per core (program-uniform; core 7 has 28 valid)
RW = 4                   # output grid rows per window
NWIN = 8                 # windows per core
WCOLS = (RW + 2) * WP    # 1482  window cols (with halo rows)
LPOS = RW * WP           # 988   conv output positions per window
WPOS = 1024              # padded positions per window
NPOS = NWIN * WPOS       # 8192
CLS_L = WPOS - 1         # cls local position in last window
NCH = 4                  # feature chunks of 128
ISL = NWIN * RW * GR     # 7840 i_shift cols per core (32 rows worth)
SQD = float(np.sqrt(D))

_cache = {}
SKIP = set()  # debug: stage names to skip in launch B


def _build_launch_a():
    nc = bacc.Bacc("TRN2", target_bir_lowering=False, debug=False, num_devices=NC)
    xa = nc.dram_tensor("xa", [4, NCH, 128, 1920], f32r, kind="ExternalInput").ap()
    wenc = nc.dram_tensor("wenc", [128, NCH], f32r, kind="ExternalInput").ap()
    benc = nc.dram_tensor("benc", [1, 1], f32, kind="ExternalInput").ap()
    a1o = nc.dram_tensor("a1o", [1, NA], f32, kind="ExternalOutput").ap()
    mxo = nc.dram_tensor("mxo", [1, 128], f32, kind="ExternalOutput").ap()
    ixo = nc.dram_tensor("ixo", [1, 128], u32, kind="ExternalOutput").ap()

    with tile.TileContext(nc) as tc:
        with tc.tile_pool(name="sb", bufs=1) as pool, \
             tc.tile_pool(name="xs", bufs=8) as xpool, \
             tc.tile_pool(name="ps", bufs=4, space="PSUM") as psp:
            went = pool.tile([128, NCH], f32r)
            nc.sync.dma_start(went[:], wenc[:])
            bent = pool.tile([1, 1], f32)
            nc.sync.dma_start(bent[:], benc[:])
            a1t = pool.tile([1, NAP], f32)
            m8t = pool.tile([1, 128], f32)
            i8t = pool.tile([1, 128], u32)
            for S in range(4):
                xts = []
                for dc in range(NCH):
                    xt = xpool.tile([128, 1920], f32r, tag="xa")
                    nc.sync.dma_start(xt[:], xa[S, dc])
                    xts.append(xt)
                for sc in range(4):
                    ps = psp.tile([1, 480], f32)
                    for dc in range(NCH):
                        nc.tensor.matmul(ps[:], went[:, dc:dc + 1],
                                         xts[dc][:, sc * 480:(sc + 1) * 480],
                                         start=(dc == 0), stop=(dc == NCH - 1))
                    o0 = S * 1920 + sc * 480
                    nc.scalar.activation(a1t[:, o0:o0 + 480], ps[:],
                                         AF.Sigmoid, bias=bent[0:1, 0:1], scale=1.0)
                    ck = S * 4 + sc
                    nc.vector.max(m8t[:, ck * 8:(ck + 1) * 8],
                                  a1t[:, o0:o0 + 480])
                    nc.vector.max_index(i8t[:, ck * 8:(ck + 1) * 8],
                                        m8t[:, ck * 8:(ck + 1) * 8],
                                        a1t[:, o0:o0 + 480])
            nc.sync.dma_start(a1o[:], a1t[:, 0:NA])
            nc.sync.dma_start(mxo[:], m8t[:])
            nc.sync.dma_start(ixo[:], i8t[:])
    nc.compile()
    return nc


# conv tap order: (a, b) row/col offsets in {-1,0,1}
TAPS = [(a, b) for a in (-1, 0, 1) for b in (-1, 0, 1)]


def _build_launch_b():
    nc = bacc.Bacc("TRN2", target_bir_lowering=False, debug=False, num_devices=NC)
    x2 = nc.dram_tensor("x2", [D, (ROWS + 3) * WP], f32, kind="ExternalInput").ap()
    cwd = nc.dram_tensor("cwd", [128, 9 * NCH * 128], bf16, kind="ExternalInput").ap()
    cbv = nc.dram_tensor("cbv", [128, NCH], f32, kind="ExternalInput").ap()
    nqv = nc.dram_tensor("nqv", [128, NCH], f32, kind="ExternalInput").ap()
    qkv = nc.dram_tensor("qkv", [128, NCH * 16], bf16, kind="ExternalInput").ap()
    qbv = nc.dram_tensor("qbv", [16, 1], f32, kind="ExternalInput").ap()
    clv = nc.dram_tensor("clv", [128, NCH], bf16, kind="ExternalInput").ap()
    mkv = nc.dram_tensor("mkv", [16, NPOS], f32, kind="ExternalInput").ap()
    eye = nc.dram_tensor("eye", [16, 16], bf16, kind="ExternalInput").ap()

    iso = nc.dram_tensor("iso", [D, NWIN * LPOS], f32, kind="ExternalOutput").ap()
    tso = nc.dram_tensor("tso", [16, D], f32, kind="ExternalOutput").ap()
    mso = nc.dram_tensor("mso", [16, 2], f32, kind="ExternalOutput").ap()

    CONV_CHUNKS = [(1, 512), (513, 474)]     # [start, width) within [0, LPOS)
    SC_CHUNKS = [(0, 512), (512, 512)]       # cover [0, WPOS)

    with tile.TileContext(nc) as tc:
        with tc.tile_pool(name="const", bufs=1) as cpool, \
             tc.tile_pool(name="work", bufs=2) as wpool, \
             tc.tile_pool(name="acc", bufs=1) as apool, \
             tc.tile_pool(name="psc", bufs=3, space="PSUM") as pscv, \
             tc.tile_pool(name="pss", bufs=2, space="PSUM") as pssc, \
             tc.tile_pool(name="pst", bufs=1, space="PSUM") as pstp, \
             tc.tile_pool(name="ptr", bufs=2, space="PSUM") as pstr:
            # constants
            cwdt = cpool.tile([128, 9 * NCH * 128], bf16)
            nc.sync.dma_start(cwdt[:], cwd[:])
            cbt = cpool.tile([128, NCH], f32)
            nc.sync.dma_start(cbt[:], cbv[:])
            nqt = cpool.tile([128, NCH], f32)
            nc.sync.dma_start(nqt[:], nqv[:])
            qkt = cpool.tile([128, NCH * 16], bf16)
            nc.sync.dma_start(qkt[:], qkv[:])
            qbt = cpool.tile([16, 1], f32)
            nc.sync.dma_start(qbt[:], qbv[:])
            eyet = cpool.tile([16, 16], bf16)
            nc.sync.dma_start(eyet[:], eye[:])

            # per-window partials
            m_all = apool.tile([16, NWIN], f32)
            s_all = apool.tile([16, NWIN], f32)
            t_all = apool.tile([16, NWIN * D], f32)
            t_acc = apool.tile([16, D], f32)

            for w in range(NWIN):
                cw0 = w * RW * WP          # x2 col of window row 0
                # 1) load x window [128, dc, WCOLS]
                xwin = wpool.tile([128, NCH, WCOLS], f32, tag="xwin", bufs=3)
                if "xload" not in SKIP:
                    nc.sync.dma_start(
                        xwin[:],
                        x2[:, cw0:cw0 + WCOLS].rearrange("(c p) l -> p c l", p=128))
                # 2) xs in bf16 (full window, for conv)
                xsb = wpool.tile([128, NCH, WCOLS], bf16, tag="xsb")
                for dc in range(NCH):
                    nc.scalar.activation(xsb[:, dc], xwin[:, dc], AF.Relu,
                                         bias=nqt[:, dc:dc + 1], scale=1.0)
                # 3) xs in fp32, in place, owned rows only (for i_shift + residual)
                for dc in range(NCH):
                    nc.vector.tensor_scalar(
                        out=xwin[:, dc, WP:WP + LPOS], in0=xwin[:, dc, WP:WP + LPOS],
                        scalar1=nqt[:, dc:dc + 1], scalar2=0.0,
                        op0=ALU.add, op1=ALU.max)
                # 4) i_shift out, padded rows (host strips pad cols)
                if "ishift" not in SKIP:
                    nc.gpsimd.dma_start(
                        iso[:, w * LPOS:(w + 1) * LPOS].rearrange(
                            "(c p) l -> p c l", p=128),
                        xwin[:, :, WP:WP + LPOS])
                # 5) conv -> xxs (bf16) ; residual from fp32 xs, +conv bias
                xxs = wpool.tile([128, NCH, WPOS], bf16, tag="xxs", bufs=3)
                nc.vector.memset(xxs[:, :, 0:1], 0.0)
                nc.vector.memset(xxs[:, :, LPOS - 1:WPOS], 0.0)
                xxa = wpool.tile([128, NCH * WPOS], bf16, tag="xxa", bufs=3)
                xxa_t = xxa[:].rearrange("p (c m k) -> p c m k", c=NCH, k=128)
                for dc in range(NCH if "conv" not in SKIP else 0):
                    for c0, cw in CONV_CHUNKS:
                        ps = pscv.tile([128, 512], f32, tag="psc")
                        for t, (a, b) in enumerate(TAPS):
                            off = WP + c0 + a * WP + b
                            nc.tensor.matmul(
                                ps[:, 0:cw],
                                cwdt[:, (t * NCH + dc) * 128:(t * NCH + dc + 1) * 128],
                                xsb[:, dc, off:off + cw],
                                start=(t == 0), stop=(t == 8))
                        nc.vector.scalar_tensor_tensor(
                            out=xxs[:, dc, c0:c0 + cw], in0=ps[:, 0:cw],
                            scalar=cbt[:, dc:dc + 1],
                            in1=xwin[:, dc, WP + c0:WP + c0 + cw],
                            op0=ALU.add, op1=ALU.add)
                    if w == NWIN - 1:
                        nc.gpsimd.dma_start(
                            xxs[:, dc:dc + 1, CLS_L:CLS_L + 1],
                            clv[:, dc:dc + 1].unsqueeze(2))
                    if "trans" not in SKIP:
                        nc.sync.dma_start(xxa_t[:, dc], xxs[:, dc, :],
                                          transpose=True)

                # 7) scores for this window (bf16) + mask
                scs = wpool.tile([16, WPOS], bf16, tag="scs", bufs=3)
                mks = wpool.tile([16, WPOS], f32, tag="mks", bufs=2)
                nc.gpsimd.dma_start(mks[:], mkv[:, w * WPOS:(w + 1) * WPOS])
                for c0, cw in SC_CHUNKS:
                    ps = pssc.tile([16, 512], f32, tag="pss")
                    for dc in range(NCH):
                        nc.tensor.matmul(ps[:], qkt[:, dc * 16:(dc + 1) * 16],
                                         xxs[:, dc, c0:c0 + cw],
                                         start=(dc == 0), stop=(dc == NCH - 1))
                    nc.vector.scalar_tensor_tensor(
                        out=scs[:, c0:c0 + cw], in0=ps[:], scalar=1.0,
                        in1=mks[:, c0:c0 + cw],
                        op0=ALU.mult, op1=ALU.add)
                # 8) independent per-window softmax partials
                m_new = wpool.tile([16, 1], f32, tag="mnew", bufs=3)
                nc.vector.tensor_reduce(m_new[:], scs[:], mybir.AxisListType.X,
                                        ALU.max)
                nc.vector.tensor_scalar(out=m_new[:], in0=m_new[:], scalar1=-1e4,
                                        scalar2=None, op0=ALU.max)
                nc.vector.tensor_copy(m_all[:, w:w + 1], m_new[:])
                ebias = wpool.tile([16, 1], f32, tag="ebias", bufs=3)
                nc.vector.tensor_tensor(out=ebias[:], in0=qbt[:], in1=m_new[:],
                                        op=ALU.subtract)
                nc.scalar.activation(scs[:], scs[:], AF.Exp, bias=ebias[:],
                                     scale=1.0, accum_out=s_all[:, w:w + 1])
                # 9) transposes: p (16 x WPOS) -> [128, 8*16]; xxs -> xxa [128, 8*512]
                pT = wpool.tile([128, (WPOS // 128) * 16], bf16, tag="pT", bufs=3)
                for blk in range(WPOS // 128):
                    pst_t = pstr.tile([128, 16], bf16, tag="ptr")
                    nc.tensor.transpose(pst_t[:], scs[:, blk * 128:(blk + 1) * 128], eyet[:])
                    nc.vector.tensor_copy(pT[:, blk * 16:(blk + 1) * 16], pst_t[:])

                # 10) T matmul for this window, accumulate with rescale
                ps = pstp.tile([16, D], f32, tag="pst")
                xxa_b = xxa[:].rearrange("p (c blk k) -> p blk c k", c=NCH, k=128)
                for blk in range(WPOS // 128):
                    nc.tensor.matmul(ps[:], pT[:, blk * 16:(blk + 1) * 16],
                                     xxa_b[:, blk],
                                     start=(blk == 0), stop=(blk == WPOS // 128 - 1))
                nc.vector.tensor_copy(t_all[:, w * D:(w + 1) * D], ps[:])

            # final combine across windows
            m_c = apool.tile([16, 1], f32)
            nc.vector.tensor_reduce(m_c[:], m_all[:], mybir.AxisListType.X,
                                    ALU.max)
            corr = apool.tile([16, NWIN], f32)
            nc.vector.tensor_scalar(out=corr[:], in0=m_all[:], scalar1=m_c[:],
                                    scalar2=None, op0=ALU.subtract)
            nc.scalar.activation(corr[:], corr[:], AF.Exp)
            sw = apool.tile([16, NWIN], f32)
            nc.vector.tensor_mul(sw[:], s_all[:], corr[:])
            s_c = apool.tile([16, 1], f32)
            nc.vector.tensor_reduce(s_c[:], sw[:], mybir.AxisListType.X, ALU.add)
            nc.vector.memset(t_acc[:], 0.0)
            for w in range(NWIN):
                nc.vector.scalar_tensor_tensor(
                    out=t_acc[:], in0=t_all[:, w * D:(w + 1) * D],
                    scalar=corr[:, w:w + 1], in1=t_acc[:],
                    op0=ALU.mult, op1=ALU.add)
            nc.sync.dma_start(tso[:], t_acc[:])
            mst = apool.tile([16, 2], f32)
            nc.vector.tensor_copy(mst[:, 0:1], m_c[:])
            nc.vector.tensor_copy(mst[:, 1:2], s_c[:])
            nc.sync.dma_start(mso[:], mst[:])
    nc.compile()
    return nc


def _grid_row_range(c):
    r0 = 31 * c
    rows = 31 if c < 7 else 28
    return r0, rows


def _host_prep_b(xT, Q, inputs, q_vec):
    """Build per-core input maps for launch B."""
    w_k = inputs["w_k"]; b_k = inputs["b_k"]
    conv_w = inputs["conv_w"]; conv_b = inputs["conv_b"]
    cls_tok = np.asarray(inputs["cls_token"]).reshape(D)

    # q heads scatter [D, 16]
    qheads = np.zeros((D, 16), np.float32)
    for h in range(NH):
        qheads[h * DH:(h + 1) * DH, h] = q_vec[h * DH:(h + 1) * DH]
    qk_full = (np.asarray(w_k, np.float64) @ qheads.astype(np.float64) / SQD
               ).astype(np.float32)                      # [D, 16]
    qb = np.zeros((16, 1), np.float32)
    for h in range(NH):
        qb[h, 0] = float(q_vec[h * DH:(h + 1) * DH].astype(np.float64)
                         @ np.asarray(b_k, np.float64)[h * DH:(h + 1) * DH] / SQD)
    # qk in SBUF layout [128, NCH*16]
    qkl = np.ascontiguousarray(
        qk_full.reshape(NCH, 128, 16).transpose(1, 0, 2).reshape(128, NCH * 16)
    ).astype(BF)

    # conv diag weights -> [9*NCH*128, 128] bf16 (no +identity; residual via STT)
    cwd = np.zeros((128, 9 * NCH, 128), np.float32)
    for t, (a, b) in enumerate(TAPS):
        wv = np.asarray(conv_w)[:, 0, a + 1, b + 1]
        for dc in range(NCH):
            blk = t * NCH + dc
            cwd[np.arange(128), blk, np.arange(128)] = wv[dc * 128:(dc + 1) * 128]
    cwd = cwd.reshape(128, 9 * NCH * 128).astype(BF)

    cb = np.asarray(conv_b, np.float32).reshape(NCH, 128).T.copy()   # [128, NCH]
    nq = (-Q).astype(np.float32).reshape(NCH, 128).T.copy()

    ins = []
    for c in range(NC):
        r0, rows = _grid_row_range(c)
        # x2: grid rows r0-1 .. r0+ROWS (inclusive), width WP with zero pads
        g = np.arange(r0 - 1, r0 + ROWS + 2)
        x2 = np.zeros((D, (ROWS + 3), WP), np.float32)
        valid = (g >= 0) & (g < GR)
        gv = g[valid]
        seq = (gv[:, None] * GR + np.arange(GR)[None, :]) % N   # [nv, 245]
        x2[:, valid, 1:246] = xT[:, seq.reshape(-1)].reshape(D, len(gv), GR)
        x2 = x2.reshape(D, (ROWS + 3) * WP)

        # mask [16, NPOS]: 0 where valid position, else -1e30
        mask = np.full((16, NPOS), -1e30, np.float32)
        for w in range(NWIN):
            for j in range(RW):
                gr = r0 + w * RW + j
                if gr < r0 + rows and gr < GR:
                    lo = w * WPOS + j * WP + 1
                    mask[0:8, lo:lo + GR] = 0.0
        if c == 0:
            mask[0:8, (NWIN - 1) * WPOS + CLS_L] = 0.0
        clsv = np.ascontiguousarray(
            (cls_tok if c == 0 else np.zeros(D, np.float32)
             ).astype(np.float32).reshape(NCH, 128).T).astype(BF)

        ins.append({"x2": x2, "cwd": cwd, "cbv": cb, "nqv": nq,
                    "qkv": qkl, "qbv": qb, "clv": clsv,
                    "mkv": mask, "eye": np.eye(16, dtype=np.float32).astype(BF)})
    return ins


def kernel(**inputs):
    if "A" not in _cache:
        _cache["A"] = _build_launch_a()
    if "B" not in _cache:
        _cache["B"] = _build_launch_b()

    x = np.ascontiguousarray(np.asarray(inputs["inputs"], np.float32))
    xT = np.ascontiguousarray(x.T)                        # [D, N]
    wenc = np.ascontiguousarray(
        np.asarray(inputs["w_enc"], np.float32).reshape(NCH, 128).T)
    benc = np.asarray(inputs["b_enc"], np.float32).reshape(1, 1)

    # ---- launch A ----
    ins_a = []
    for c in range(NC):
        xa = np.empty((D, NAP), np.float32)
        xa[:, :NA] = xT[:, c * NA:(c + 1) * NA]
        xa[:, NA:] = xa[:, 0:1]
        # interleave to [S=4, NCH, 128, 1920]
        xa = np.ascontiguousarray(
            xa.reshape(NCH, 128, 4, 1920).transpose(2, 0, 1, 3))
        ins_a.append({"xa": xa, "wenc": wenc, "benc": benc})
    res_a = run_bass_kernel_spmd(_cache["A"], ins_a, core_ids=list(range(NC)))
    _cache["res_A"] = res_a
    a1 = np.concatenate([res_a.results[c]["a1o"][0] for c in range(NC)])
    idx = None
    best = -np.inf
    for c in range(NC):
        mx = res_a.results[c]["mxo"][0]          # [128] = 16 chunks x top8
        ix = res_a.results[c]["ixo"][0]
        for ck in range(16):
            v = float(mx[ck * 8])
            if v > best:
                S, sc = divmod(ck, 4)
                li = int(ix[ck * 8])
                g = S * 1920 + sc * 480 + li
                if g >= NA:
                    g = 0                        # padding duplicates column 0
                best = v
                idx = c * NA + g
    Q = x[idx].astype(np.float32)

    # ---- launch B ----
    w_q = np.asarray(inputs["w_q"], np.float64)
    b_q = np.asarray(inputs["b_q"], np.float64)
    q_vec = (Q.astype(np.float64) @ w_q + b_q).astype(np.float32)
    ins_b = _host_prep_b(xT, Q, inputs, q_vec)
    res_b = run_bass_kernel_spmd(_cache["B"], ins_b, core_ids=list(range(NC)))
    _cache["res_B"] = res_b

    # ---- host combine ----
    i_shift = np.empty((N, D), np.float32)
    for c in range(NC):
        r0, rows = _grid_row_range(c)
        s0 = r0 * GR
        v = min(rows * GR, N - s0)
        iso = res_b.results[c]["iso"].reshape(D, NWIN * RW, WP)[:, :, 1:246]
        i_shift[s0:s0 + v] = iso.reshape(D, NWIN * RW * GR)[:, :v].T

    m_c = np.stack([res_b.results[c]["mso"][0:8, 0] for c in range(NC)])   # [NC, 8]
    s_c = np.stack([res_b.results[c]["mso"][0:8, 1] for c in range(NC)])
    t_c = np.stack([res_b.results[c]["tso"][0:8, :] for c in range(NC)])   # [NC, 8, D]

    m_g = m_c.max(axis=0)                                  # [8]
    wgt = np.exp(m_c.astype(np.float64) - m_g[None, :])    # [NC, 8]
    S = (wgt * s_c).sum(axis=0)                            # [8]
    T = (wgt[:, :, None] * t_c).sum(axis=0)                # [8, D]

    w_v = np.asarray(inputs["w_v"], np.float64)
    b_v = np.asarray(inputs["b_v"], np.float64)
    O = np.empty(D, np.float64)
    for h in range(NH):
        pv = T[h] @ w_v[:, h * DH:(h + 1) * DH] + S[h] * b_v[h * DH:(h + 1) * DH]
        O[h * DH:(h + 1) * DH] = q_vec[h * DH:(h + 1) * DH] + pv / S[h]

    w_o = np.asarray(inputs["w_o"], np.float64)
    b_o = np.asarray(inputs["b_o"], np.float64)
    O2 = O + np.maximum(O @ w_o + b_o, 0.0)
    w_fc = np.asarray(inputs["w_fc"], np.float64)
    b_fc = np.asarray(inputs["b_fc"], np.float64)
    out = (O2 @ w_fc + b_fc).astype(np.float32)[None, :]   # [1, 2]

    return out, i_shift[None], a1[None].astype(np.float32)


# revision 40
# speedup vs baseline: 1.0028x; 1.0028x over previous
"""Trainium2 Bass kernel for nn_FRMIL (dense_transformer, 8 NeuronCores).

Pipeline (2 SPMD launches over 8 cores):
  Launch A: per-core enc scoring a1 = sigmoid(x @ w_enc + b_enc), top-1 value+index.
            Host picks the global argmax row -> Q.
  Launch B: per-core (grid-row sharded, sequence-parallel):
            xs = relu(x - Q)  [fp32, streamed out as i_shift]
            depthwise 3x3 conv over the 245x245 grid (+bias+residual) -> xx (bf16)
            scores_h = (w_k @ q_heads).T @ xx / sqrt(D) (+ q.b_k)
            online softmax over positions: m, sumexp, T = sum_s p_s * xx[:, s]
  Host: combines per-core (m, sumexp, T) partials, applies the tiny w_v/w_o/w_fc
        tail (~1 MFLOP), assembles full outputs.

Layout: "B" = feature-on-partition [512 -> 4 chunks of 128, positions on free axis].
Grid rows padded to width 247 (zero col left/right) so conv col-taps are free-axis
shifts; grid-row halo rows come via overlapping DMA windows.
"""
import numpy as np
import ml_dtypes

import concourse.bacc as bacc
import concourse.bass as bass
import concourse.tile as tile
import concourse.mybir as mybir
from concourse.bass_utils import run_bass_kernel_spmd

f32 = mybir.dt.float32
f32r = mybir.dt.float32r
bf16 = mybir.dt.bfloat16
u32 = mybir.dt.uint32
AF = mybir.ActivationFunctionType
ALU = mybir.AluOpType
BF = ml_dtypes.bfloat16

N, D, NH, DH, NCLS = 60000, 512, 8, 64, 2
GR = 245                 # grid rows/cols
WP = 247                 # padded row width
NC = 8                   # cores
NA = N // NC             # 7500 rows/core for launch A
NAP = 7680               # padded to 15*512
ROWS = 31                # grid rows per core (program-uniform; core 7 has 28 valid)
RW = 4                   # output grid rows per window
NWIN = 8                 # windows per core
WCOLS = (RW + 2) * WP    # 1482  window cols (with halo rows)
LPOS = RW * WP           # 988   conv output positions per window
WPOS = 1024              # padded positions per window
NPOS = NWIN * WPOS       # 8192
CLS_L = WPOS - 1         # cls local position in last window
NCH = 4                  # feature chunks of 128
ISL = NWIN * RW * GR     # 7840 i_shift cols per core (32 rows worth)
SQD = float(np.sqrt(D))

_cache = {}
SKIP = set()  # debug: stage names to skip in launch B


def _build_launch_a():
    nc = bacc.Bacc("TRN2", target_bir_lowering=False, debug=False, num_devices=NC)
    xa = nc.dram_tensor("xa", [4, NCH, 128, 1920], f32r, kind="ExternalInput").ap()
    wenc = nc.dram_tensor("wenc", [128, NCH], f32r, kind="ExternalInput").ap()
    benc = nc.dram_tensor("benc", [1, 1], f32, kind="ExternalInput").ap()
    a1o = nc.dram_tensor("a1o", [1, NA], f32, kind="ExternalOutput").ap()
    mxo = nc.dram_tensor("mxo", [1, 128], f32, kind="ExternalOutput").ap()
    ixo = nc.dram_tensor("ixo", [1, 128], u32, kind="ExternalOutput").ap()

    with tile.TileContext(nc) as tc:
        with tc.tile_pool(name="sb", bufs=1) as pool, \
             tc.tile_pool(name="xs", bufs=8) as xpool, \
             tc.tile_pool(name="ps", bufs=4, space="PSUM") as psp:
            went = pool.tile([128, NCH], f32r)
            nc.sync.dma_start(went[:], wenc[:])
            bent = pool.tile([1, 1], f32)
            nc.sync.dma_start(bent[:], benc[:])
            a1t = pool.tile([1, NAP], f32)
            m8t = pool.tile([1, 128], f32)
            i8t = pool.tile([1, 128], u32)
            for S in range(4):
                xts = []
                for dc in range(NCH):
                    xt = xpool.tile([128, 1920], f32r, tag="xa")
                    nc.sync.dma_start(xt[:], xa[S, dc])
                    xts.append(xt)
                for sc in range(4):
                    ps = psp.tile([1, 480], f32)
                    for dc in range(NCH):
                        nc.tensor.matmul(ps[:], went[:, dc:dc + 1],
                                         xts[dc][:, sc * 480:(sc + 1) * 480],
                                         start=(dc == 0), stop=(dc == NCH - 1))
                    o0 = S * 1920 + sc * 480
                    nc.scalar.activation(a1t[:, o0:o0 + 480], ps[:],
                                         AF.Sigmoid, bias=bent[0:1, 0:1], scale=1.0)
                    ck = S * 4 + sc
                    nc.vector.max(m8t[:, ck * 8:(ck + 1) * 8],
                                  a1t[:, o0:o0 + 480])
                    nc.vector.max_index(i8t[:, ck * 8:(ck + 1) * 8],
                                        m8t[:, ck * 8:(ck + 1) * 8],
                                        a1t[:, o0:o0 + 480])
            nc.sync.dma_start(a1o[:], a1t[:, 0:NA])
            nc.sync.dma_start(mxo[:], m8t[:])
            nc.sync.dma_start(ixo[:], i8t[:])
    nc.compile()
    return nc


# conv tap order: (a, b) row/col offsets in {-1,0,1}
TAPS = [(a, b) for a in (-1, 0, 1) for b in (-1, 0, 1)]


def _build_launch_b():
    nc = bacc.Bacc("TRN2", target_bir_lowering=False, debug=False, num_devices=NC)
    x2 = nc.dram_tensor("x2", [D, (ROWS + 3) * WP], f32, kind="ExternalInput").ap()
    cwd = nc.dram_tensor("cwd", [128, 9 * NCH * 128], bf16, kind="ExternalInput").ap()
    cbv = nc.dram_tensor("cbv", [128, NCH], f32, kind="ExternalInput").ap()
    nqv = nc.dram_tensor("nqv", [128, NCH], f32, kind="ExternalInput").ap()
    qkv = nc.dram_tensor("qkv", [128, NCH * 16], bf16, kind="ExternalInput").ap()
    qbv = nc.dram_tensor("qbv", [16, 1], f32, kind="ExternalInput").ap()
    clv = nc.dram_tensor("clv", [128, NCH], bf16, kind="ExternalInput").ap()
    mkv = nc.dram_tensor("mkv", [16, NPOS], f32, kind="ExternalInput").ap()
    eye = nc.dram_tensor("eye", [16, 16], bf16, kind="ExternalInput").ap()

    iso = nc.dram_tensor("iso", [D, NWIN * LPOS], f32, kind="ExternalOutput").ap()
    tso = nc.dram_tensor("tso", [16, D], f32, kind="ExternalOutput").ap()
    mso = nc.dram_tensor("mso", [16, 2], f32, kind="ExternalOutput").ap()

    CONV_CHUNKS = [(1, 512), (513, 474)]     # [start, width) within [0, LPOS)
    SC_CHUNKS = [(0, 512), (512, 512)]       # cover [0, WPOS)

    with tile.TileContext(nc) as tc:
        with tc.tile_pool(name="const", bufs=1) as cpool, \
             tc.tile_pool(name="work", bufs=2) as wpool, \
             tc.tile_pool(name="acc", bufs=1) as apool, \
             tc.tile_pool(name="psc", bufs=3, space="PSUM") as pscv, \
             tc.tile_pool(name="pss", bufs=1, space="PSUM") as pssc, \
             tc.tile_pool(name="pst", bufs=2, space="PSUM") as pstp, \
             tc.tile_pool(name="ptr", bufs=2, space="PSUM") as pstr:
            # constants
            cwdt = cpool.tile([128, 9 * NCH * 128], bf16)
            nc.sync.dma_start(cwdt[:], cwd[:])
            cbt = cpool.tile([128, NCH], f32)
            nc.sync.dma_start(cbt[:], cbv[:])
            nqt = cpool.tile([128, NCH], f32)
            nc.sync.dma_start(nqt[:], nqv[:])
            qkt = cpool.tile([128, NCH * 16], bf16)
            nc.sync.dma_start(qkt[:], qkv[:])
            qbt = cpool.tile([16, 1], f32)
            nc.sync.dma_start(qbt[:], qbv[:])
            eyet = cpool.tile([16, 16], bf16)
            nc.sync.dma_start(eyet[:], eye[:])

            # per-window partials
            m_all = apool.tile([16, NWIN], f32)
            s_all = apool.tile([16, NWIN], f32)
            t_all = apool.tile([16, NWIN * D], f32)
            t_acc = apool.tile([16, D], f32)

            for w in range(NWIN):
                cw0 = w * RW * WP          # x2 col of window row 0
                # 1) load x window [128, dc, WCOLS]
                xwin = wpool.tile([128, NCH, WCOLS], f32, tag="xwin", bufs=3)
                if "xload" not in SKIP:
                    nc.sync.dma_start(
                        xwin[:],
                        x2[:, cw0:cw0 + WCOLS].rearrange("(c p) l -> p c l", p=128))
                # 2) xs in bf16 (full window, for conv)
                xsb = wpool.tile([128, NCH, WCOLS], bf16, tag="xsb")
                for dc in range(NCH):
                    nc.scalar.activation(xsb[:, dc], xwin[:, dc], AF.Relu,
                                         bias=nqt[:, dc:dc + 1], scale=1.0)
                # 3) xs in fp32, in place, owned rows only (for i_shift + residual)
                for dc in range(NCH):
                    nc.vector.tensor_scalar(
                        out=xwin[:, dc, WP:WP + LPOS], in0=xwin[:, dc, WP:WP + LPOS],
                        scalar1=nqt[:, dc:dc + 1], scalar2=0.0,
                        op0=ALU.add, op1=ALU.max)
                # 4) i_shift out, padded rows (host strips pad cols)
                if "ishift" not in SKIP:
                    nc.gpsimd.dma_start(
                        iso[:, w * LPOS:(w + 1) * LPOS].rearrange(
                            "(c p) l -> p c l", p=128),
                        xwin[:, :, WP:WP + LPOS])
                # 5) conv -> xxs (bf16) ; residual from fp32 xs, +conv bias
                xxs = wpool.tile([128, NCH, WPOS], bf16, tag="xxs", bufs=3)
                nc.vector.memset(xxs[:, :, 0:1], 0.0)
                nc.vector.memset(xxs[:, :, LPOS - 1:WPOS], 0.0)
                xxa = wpool.tile([128, NCH * WPOS], bf16, tag="xxa", bufs=3)
                xxa_t = xxa[:].rearrange("p (c m k) -> p c m k", c=NCH, k=128)
                for dc in range(NCH if "conv" not in SKIP else 0):
                    for c0, cw in CONV_CHUNKS:
                        ps = pscv.tile([128, 512], f32, tag="psc")
                        for t, (a, b) in enumerate(TAPS):
                            off = WP + c0 + a * WP + b
                            nc.tensor.matmul(
                                ps[:, 0:cw],
                                cwdt[:, (t * NCH + dc) * 128:(t * NCH + dc + 1) * 128],
                                xsb[:, dc, off:off + cw],
                                start=(t == 0), stop=(t == 8))
                        nc.vector.scalar_tensor_tensor(
                            out=xxs[:, dc, c0:c0 + cw], in0=ps[:, 0:cw],
                            scalar=cbt[:, dc:dc + 1],
                            in1=xwin[:, dc, WP + c0:WP + c0 + cw],
                            op0=ALU.add, op1=ALU.add)
                    if w == NWIN - 1:
                        nc.gpsimd.dma_start(
                            xxs[:, dc:dc + 1, CLS_L:CLS_L + 1],
                            clv[:, dc:dc + 1].unsqueeze(2))
                    if "trans" not in SKIP:
                        nc.sync.dma_start(xxa_t[:, dc], xxs[:, dc, :],
                                          transpose=True)

                # 7) scores for this window (bf16) + mask
                scs = wpool.tile([16, WPOS], bf16, tag="scs", bufs=3)
                mks = wpool.tile([16, WPOS], f32, tag="mks", bufs=2)
                nc.gpsimd.dma_start(mks[:], mkv[:, w * WPOS:(w + 1) * WPOS])
                for c0, cw in SC_CHUNKS:
                    ps = pssc.tile([16, 512], f32, tag="pss")
                    for dc in range(NCH):
                        nc.tensor.matmul(ps[:], qkt[:, dc * 16:(dc + 1) * 16],
                                         xxs[:, dc, c0:c0 + cw],
                                         start=(dc == 0), stop=(dc == NCH - 1))
                    nc.vector.scalar_tensor_tensor(
                        out=scs[:, c0:c0 + cw], in0=ps[:], scalar=1.0,
                        in1=mks[:, c0:c0 + cw],
                        op0=ALU.mult, op1=ALU.add)
                # 8) independent per-window softmax partials
                m_new = wpool.tile([16, 1], f32, tag="mnew", bufs=3)
                nc.vector.tensor_reduce(m_new[:], scs[:], mybir.AxisListType.X,
                                        ALU.max)
                nc.vector.tensor_scalar(out=m_new[:], in0=m_new[:], scalar1=-1e4,
                                        scalar2=None, op0=ALU.max)
                nc.vector.tensor_copy(m_all[:, w:w + 1], m_new[:])
                ebias = wpool.tile([16, 1], f32, tag="ebias", bufs=3)
                nc.vector.tensor_tensor(out=ebias[:], in0=qbt[:], in1=m_new[:],
                                        op=ALU.subtract)
                nc.scalar.activation(scs[:], scs[:], AF.Exp, bias=ebias[:],
                                     scale=1.0, accum_out=s_all[:, w:w + 1])
                # 9) transposes: p (16 x WPOS) -> [128, 8*16]; xxs -> xxa [128, 8*512]
                pT = wpool.tile([128, (WPOS // 128) * 16], bf16, tag="pT", bufs=3)
                for blk in range(WPOS // 128):
                    pst_t = pstr.tile([128, 16], bf16, tag="ptr")
                    nc.tensor.transpose(pst_t[:], scs[:, blk * 128:(blk + 1) * 128], eyet[:])
                    nc.vector.tensor_copy(pT[:, blk * 16:(blk + 1) * 16], pst_t[:])

                # 10) T matmul for this window, accumulate with rescale
                ps = pstp.tile([16, D], f32, tag="pst")
                xxa_b = xxa[:].rearrange("p (c blk k) -> p blk c k", c=NCH, k=128)
                for blk in range(WPOS // 128):
                    nc.tensor.matmul(ps[:], pT[:, blk * 16:(blk + 1) * 16],
                                     xxa_b[:, blk],
                                     start=(blk == 0), stop=(blk == WPOS // 128 - 1))
                nc.vector.tensor_copy(t_all[:, w * D:(w + 1) * D], ps[:])

            # final combine across windows
            m_c = apool.tile([16, 1], f32)
            nc.vector.tensor_reduce(m_c[:], m_all[:], mybir.AxisListType.X,
                                    ALU.max)
            corr = apool.tile([16, NWIN], f32)
            nc.vector.tensor_scalar(out=corr[:], in0=m_all[:], scalar1=m_c[:],
                                    scalar2=None, op0=ALU.subtract)
            nc.scalar.activation(corr[:], corr[:], AF.Exp)
            sw = apool.tile([16, NWIN], f32)
            nc.vector.tensor_mul(sw[:], s_all[:], corr[:])
            s_c = apool.tile([16, 1], f32)
            nc.vector.tensor_reduce(s_c[:], sw[:], mybir.AxisListType.X, ALU.add)
            nc.vector.memset(t_acc[:], 0.0)
            for w in range(NWIN):
                nc.vector.scalar_tensor_tensor(
                    out=t_acc[:], in0=t_all[:, w * D:(w + 1) * D],
                    scalar=corr[:, w:w + 1], in1=t_acc[:],
                    op0=ALU.mult, op1=ALU.add)
            nc.sync.dma_start(tso[:], t_acc[:])
            mst = apool.tile([16, 2], f32)
            nc.vector.tensor_copy(mst[:, 0:1], m_c[:])
            nc.vector.tensor_copy(mst[:, 1:2], s_c[:])
            nc.sync.dma_start(mso[:], mst[:])
    nc.compile()
    return nc


def _grid_row_range(c):
    r0 = 31 * c
    rows = 31 if c < 7 else 28
    return r0, rows


def _host_prep_b(xT, Q, inputs, q_vec):
    """Build per-core input maps for launch B."""
    w_k = inputs["w_k"]; b_k = inputs["b_k"]
    conv_w = inputs["conv_w"]; conv_b = inputs["conv_b"]
    cls_tok = np.asarray(inputs["cls_token"]).reshape(D)

    # q heads scatter [D, 16]
    qheads = np.zeros((D, 16), np.float32)
    for h in range(NH):
        qheads[h * DH:(h + 1) * DH, h] = q_vec[h * DH:(h + 1) * DH]
    qk_full = (np.asarray(w_k, np.float64) @ qheads.astype(np.float64) / SQD
               ).astype(np.float32)                      # [D, 16]
    qb = np.zeros((16, 1), np.float32)
    for h in range(NH):
        qb[h, 0] = float(q_vec[h * DH:(h + 1) * DH].astype(np.float64)
                         @ np.asarray(b_k, np.float64)[h * DH:(h + 1) * DH] / SQD)
    # qk in SBUF layout [128, NCH*16]
    qkl = np.ascontiguousarray(
        qk_full.reshape(NCH, 128, 16).transpose(1, 0, 2).reshape(128, NCH * 16)
    ).astype(BF)

    # conv diag weights -> [9*NCH*128, 128] bf16 (no +identity; residual via STT)
    cwd = np.zeros((128, 9 * NCH, 128), np.float32)
    for t, (a, b) in enumerate(TAPS):
        wv = np.asarray(conv_w)[:, 0, a + 1, b + 1]
        for dc in range(NCH):
            blk = t * NCH + dc
            cwd[np.arange(128), blk, np.arange(128)] = wv[dc * 128:(dc + 1) * 128]
    cwd = cwd.reshape(128, 9 * NCH * 128).astype(BF)

    cb = np.asarray(conv_b, np.float32).reshape(NCH, 128).T.copy()   # [128, NCH]
    nq = (-Q).astype(np.float32).reshape(NCH, 128).T.copy()

    ins = []
    for c in range(NC):
        r0, rows = _grid_row_range(c)
        # x2: grid rows r0-1 .. r0+ROWS (inclusive), width WP with zero pads
        g = np.arange(r0 - 1, r0 + ROWS + 2)
        x2 = np.zeros((D, (ROWS + 3), WP), np.float32)
        valid = (g >= 0) & (g < GR)
        gv = g[valid]
        seq = (gv[:, None] * GR + np.arange(GR)[None, :]) % N   # [nv, 245]
        x2[:, valid, 1:246] = xT[:, seq.reshape(-1)].reshape(D, len(gv), GR)
        x2 = x2.reshape(D, (ROWS + 3) * WP)

        # mask [16, NPOS]: 0 where valid position, else -1e30
        mask = np.full((16, NPOS), -1e30, np.float32)
        for w in range(NWIN):
            for j in range(RW):
                gr = r0 + w * RW + j
                if gr < r0 + rows and gr < GR:
                    lo = w * WPOS + j * WP + 1
                    mask[0:8, lo:lo + GR] = 0.0
        if c == 0:
            mask[0:8, (NWIN - 1) * WPOS + CLS_L] = 0.0
        clsv = np.ascontiguousarray(
            (cls_tok if c == 0 else np.zeros(D, np.float32)
             ).astype(np.float32).reshape(NCH, 128).T).astype(BF)

        ins.append({"x2": x2, "cwd": cwd, "cbv": cb, "nqv": nq,
                    "qkv": qkl, "qbv": qb, "clv": clsv,
                    "mkv": mask, "eye": np.eye(16, dtype=np.float32).astype(BF)})
    return ins


def kernel(**inputs):
    if "A" not in _cache:
        _cache["A"] = _build_launch_a()
    if "B" not in _cache:
        _cache["B"] = _build_launch_b()

    x = np.ascontiguousarray(np.asarray(inputs["inputs"], np.float32))
    xT = np.ascontiguousarray(x.T)                        # [D, N]
    wenc = np.ascontiguousarray(
        np.asarray(inputs["w_enc"], np.float32).reshape(NCH, 128).T)
    benc = np.asarray(inputs["b_enc"], np.float32).reshape(1, 1)

    # ---- launch A ----
    ins_a = []
    for c in range(NC):
        xa = np.empty((D, NAP), np.float32)
        xa[:, :NA] = xT[:, c * NA:(c + 1) * NA]
        xa[:, NA:] = xa[:, 0:1]
        # interleave to [S=4, NCH, 128, 1920]
        xa = np.ascontiguousarray(
            xa.reshape(NCH, 128, 4, 1920).transpose(2, 0, 1, 3))
        ins_a.append({"xa": xa, "wenc": wenc, "benc": benc})
    res_a = run_bass_kernel_spmd(_cache["A"], ins_a, core_ids=list(range(NC)))
    _cache["res_A"] = res_a
    a1 = np.concatenate([res_a.results[c]["a1o"][0] for c in range(NC)])
    idx = None
    best = -np.inf
    for c in range(NC):
        mx = res_a.results[c]["mxo"][0]          # [128] = 16 chunks x top8
        ix = res_a.results[c]["ixo"][0]
        for ck in range(16):
            v = float(mx[ck * 8])
            if v > best:
                S, sc = divmod(ck, 4)
                li = int(ix[ck * 8])
                g = S * 1920 + sc * 480 + li
                if g >= NA:
                    g = 0                        # padding duplicates column 0
                best = v
                idx = c * NA + g
    Q = x[idx].astype(np.float32)

    # ---- launch B ----
    w_q = np.asarray(inputs["w_q"], np.float64)
    b_q = np.asarray(inputs["b_q"], np.float64)
    q_vec = (Q.astype(np.float64) @ w_q + b_q).astype(np.float32)
    ins_b = _host_prep_b(xT, Q, inputs, q_vec)
    res_b = run_bass_kernel_spmd(_cache["B"], ins_b, core_ids=list(range(NC)))
    _cache["res_B"] = res_b

    # ---- host combine ----
    i_shift = np.empty((N, D), np.float32)
    for c in range(NC):
        r0, rows = _grid_row_range(c)
        s0 = r0 * GR
        v = min(rows * GR, N - s0)
        iso = res_b.results[c]["iso"].reshape(D, NWIN * RW, WP)[:, :, 1:246]
        i_shift[s0:s0 + v] = iso.reshape(D, NWIN * RW * GR)[:, :v].T

    m_c = np.stack([res_b.results[c]["mso"][0:8, 0] for c in range(NC)])   # [NC, 8]
    s_c = np.stack([res_b.results[c]["mso"][0:8, 1] for c in range(NC)])
    t_c = np.stack([res_b.results[c]["tso"][0:8, :] for c in range(NC)])   # [NC, 8, D]

    m_g = m_c.max(axis=0)                                  # [8]
    wgt = np.exp(m_c.astype(np.float64) - m_g[None, :])    # [NC, 8]
    S = (wgt * s_c).sum(axis=0)                            # [8]
    T = (wgt[:, :, None] * t_c).sum(axis=0)                # [8, D]

    w_v = np.asarray(inputs["w_v"], np.float64)
    b_v = np.asarray(inputs["b_v"], np.float64)
    O = np.empty(D, np.float64)
    for h in range(NH):
        pv = T[h] @ w_v[:, h * DH:(h + 1) * DH] + S[h] * b_v[h * DH:(h + 1) * DH]
        O[h * DH:(h + 1) * DH] = q_vec[h * DH:(h + 1) * DH] + pv / S[h]

    w_o = np.asarray(inputs["w_o"], np.float64)
    b_o = np.asarray(inputs["b_o"], np.float64)
    O2 = O + np.maximum(O @ w_o + b_o, 0.0)
    w_fc = np.asarray(inputs["w_fc"], np.float64)
    b_fc = np.asarray(inputs["b_fc"], np.float64)
    out = (O2 @ w_fc + b_fc).astype(np.float32)[None, :]   # [1, 2]

    return out, i_shift[None], a1[None].astype(np.float32)
